# revision 62
# baseline (speedup 1.0000x reference)
"""Trainium2 Bass kernel for nn_MoE_AllToAll_Layer (top-1 MoE, 8 experts).

Expert parallel across 8 NeuronCores: core e holds expert e's FFN weights.
Core e computes the router for ITS OWN 512-token slice only (2MB fp32 x
stream instead of a replicated 16MB one), broadcasts the per-token
(argmax-code, score) pairs to all cores with one small AllGather, then each
core rebuilds the full routing state and runs the counting-sort / scatter /
gather / FFN pipeline:
 - stable counting sort of all 4096 tokens by expert, scatter of
   (token_id, score) records into a sorted-position-indexed DRAM array with
   ONE dma_scatter_add, gather of the core's own expert rows with
   dma_gather, expert FFN on the compacted tokens, compact scaled output
   rows + token ids written out; the host places rows back by token id
   (pure data movement).

Design notes:
 - The router is exact fp32 (an fp16 router flips ~1 argmax on these inputs,
   and one flip shifts the reference's positional score permutation, which
   corrupts hundreds of rows). x is streamed fp32 but used as the PE's
   stationary operand with the tiny Wr moving, so fp32's 4 cycles/row apply
   only to 8-wide outputs. Only the 512-token slice is routed per core.
 - argmax code renc = E - argmax in ONE reduction: max over e of
   (lg*2^20 + (E-e)) minus max(lg)*2^20. Exact: lg*2^20 is a power-of-two
   scale, the +-int offsets are representable, and the final subtract is of
   two nearby fp32s. The priority only flips selections when a non-max
   logit is within 8*2^-20 of the max; observed top-2 gaps are > 2e-4.
 - AllGather payload: pk[p, 0:4]=renc, pk[p, 4:8]=score (fp16: renc is an
   exact small int, score is applied in fp16 at the diag step anyway) is
   PE-transposed to [8, 128] rows, AllGather concatenates cores to
   [64, 128], and two static selector matmuls ([64, 32] one-hots) transpose
   renc/score back to [P, NT] in one shot.
 - The counting sort's carry (expert base offset + within-expert exclusive
   prefix over tiles) is TWO PE matmuls: colsumT = m16half^T @ ones gives
   per-(tile,expert) counts on partitions; carry = broadcast(colsumT) @ W
   with the static 0/1 matrix W[(t'e'),(te)] = [e'<e] + [e'==e][t'<t].
   This replaces the serial Hillis-Steele scan chains. The core's own base
   offset comes from the same broadcast via a per-core [e'<e_core] column.
 - dma_gather/dma_scatter_add index layout: [128, n/16] int16, the
   [16, n/16] wrap (slot i at [i%16, i//16]) replicated 8x down partitions
   (one copy per Q7 core); built on the PE with 8 selection matmuls.
   Gather writes slot i to out[i%128, i//128].
 - The reference's positional score scale is folded into the PE transpose
   of the gathered x rows via a plain matmul with diag(score) as the rhs.
 - FFN weights/activations/outputs are fp16 (1 cycle/row, half the DMA).
   Layer-1 streams W1, layer-2 accumulates all 32 hidden chunks in PSUM.
 - All DMAs contend on ONE serial DMA_ENGINES resource (FIFO), so the
   16MB W1/W2 stream is PACED: each 1MB chunk's DMA is gated on the
   previous chunk via a corner copy, with the whole chain gated to start
   after the router payload is packed and to pause until the xg gather.
   This keeps at most one weight chunk in flight whenever the small
   latency-critical scatter/gather DMAs need the bus.
"""

import numpy as np
import sys

sys.path.insert(0, "/opt/trn_rl_repo")

import concourse.bass as bass  # noqa: E402
import concourse.tile as tile  # noqa: E402
from concourse import bacc, mybir  # noqa: E402
from concourse.bass_utils import run_bass_kernel_spmd  # noqa: E402

P = 128
N_TOKENS = 4096
D_IN = 1024
D_HID = 4096
D_OUT = 1024
E = 8
NT = N_TOKENS // P          # 32 token tiles
DC = D_IN // P              # 8 d-chunks
KC = D_OUT // P             # 8 k-chunks
JG = D_HID // P             # 32 hidden chunks
CAP = 640                   # gather/scatter slot capacity (5 row tiles)
FCAP = 608                  # FFN compute width: expert counts are platform-
                            # dependent (536 max on cpu-generated inputs, 583
                            # on device-generated ones); counts are
                            # ~Binomial(4096, 1/8) so P(count > 608) ~ 2e-5
RT = CAP // P               # 5 row tiles
BT = N_TOKENS // E          # 512 tokens per core slice
TPB = BT // P               # 4 token tiles per slice
NJB = 8                     # W1 streaming blocks (512 hidden each)
JB = D_HID // NJB           # 512
JCB = JB // P               # 4 hidden chunks per W1 block
NWB = 8                     # W2 streaming chunks (4 hidden chunks each)
REC = 64                    # f32 row stride of sidx records (256B min)
PSCALE = float(2 ** 20)     # renc priority scale

dt = mybir.dt
Alu = mybir.AluOpType
Act = mybir.ActivationFunctionType
Ax = mybir.AxisListType

f32 = dt.float32
f16 = dt.float16
i16 = dt.int16

# c16 const blob column offsets (fp16)
C16_IDENT = 0            # [0:128)   eye(128)
C16_TRI = 128            # [128:256) tri[q,p] = q < p
C16_ONES = 256           # [256:257) 1.0 column
C16_SELR = 257           # [257:289) renc-row selector [64, 32]
C16_SELS = 289           # [289:321) score-row selector [64, 32]
C16_O128 = 321           # [321:449) all-ones [128, 128]
C16_WA = 449             # [449:705) carry matrix, t' in 0..16
C16_WB = 705             # [705:961) carry matrix, t' in 16..32
C16_EXP8 = 961           # [961:969) expert one-hot [k%8 == e]
C16_W = 969
# cf32 const blob column offsets (fp32)
CF_IOTAC = 0             # [0:32)  iotac[p, t] = t*128 + p
CF_IOTAW = 32            # [32:72) iotaw[p, m] = 16*m + p%16 (wrapped iota)
CF_WR = 72               # [72:136) wr32[p, c*8+e] = Wr[c*128+p, e] (fp32!)
CF_IOTAE = 136           # [136:168) iotaE[p, t*8+e] = E - e
CF_W = 168


def build_nc():
    nc = bacc.Bacc(
        "TRN2",
        target_bir_lowering=False,
        debug=False,
        enable_asserts=False,
        num_devices=E,
    )

    # per-core router slice: x32s[p, c, n] = x[e*512 + n, c*128 + p]
    x32s = nc.dram_tensor("x32s", [P, DC, BT], f32, kind="ExternalInput").ap()
    x16 = nc.dram_tensor("x16", [N_TOKENS, D_IN], f16, kind="ExternalInput").ap()
    w1t = nc.dram_tensor("w1t", [P, NJB, DC, JB], f16, kind="ExternalInput").ap()
    w2t = nc.dram_tensor("w2t", [P, NWB, JG // NWB, D_OUT], f16, kind="ExternalInput").ap()
    c16 = nc.dram_tensor("c16", [P, C16_W], f16, kind="ExternalInput").ap()
    cf32 = nc.dram_tensor("cf32", [P, CF_W], f32, kind="ExternalInput").ap()
    # per-core [e' < e_core] indicator on (t'e') partition rows
    crp = nc.dram_tensor("crp", [P, 1], f16, kind="ExternalInput").ap()
    # pre-zeroed scatter destination (host ships zeros)
    sidx = nc.dram_tensor("sidx", [N_TOKENS, REC], f32, kind="ExternalInput").ap()
    # wrap/replicate selector: selg[p, g, q] = 1 iff p == 16*g + (q % 16)
    selg = nc.dram_tensor("selg", [P, 8, P], f32, kind="ExternalInput").ap()

    outT16 = nc.dram_tensor("outT16", [P, KC, FCAP], f16, kind="ExternalOutput").ap()
    ids5 = nc.dram_tensor("ids5", [P, RT], f32, kind="ExternalOutput").ap()
    cnts = nc.dram_tensor("cnts", [1, E], f32, kind="ExternalOutput").ap()

    with tile.TileContext(nc) as tc:
        emit(nc, tc, locals())
    nc.compile()
    return nc


def emit(nc, tc, io):
    x32s, x16, w1t, w2t = io["x32s"], io["x16"], io["w1t"], io["w2t"]
    c16, cf32, crp = io["c16"], io["cf32"], io["crp"]
    outT16, ids5, cnts = io["outT16"], io["ids5"], io["cnts"]
    sidx = io["sidx"]

    with tc.tile_pool(name="consts", bufs=1) as cpool:
        cf_sb = cpool.tile([P, CF_W], f32, tag="cf32")
        nc.sync.dma_start(cf_sb[:], cf32)
        c16_sb = cpool.tile([P, C16_W], f16, tag="c16")
        crp_sb = cpool.tile([P, 1], f16, tag="crp")
        selg_sb = cpool.tile([P, 8, P], f32, tag="selg")

        ident16 = c16_sb[:, C16_IDENT:C16_IDENT + P]
        tri16 = c16_sb[:, C16_TRI:C16_TRI + P]
        ones1_16 = c16_sb[:, C16_ONES:C16_ONES + 1]
        sel_r = c16_sb[0:64, C16_SELR:C16_SELR + NT]
        sel_s = c16_sb[0:64, C16_SELS:C16_SELS + NT]
        ones128 = c16_sb[:, C16_O128:C16_O128 + P]
        w_carA = c16_sb[:, C16_WA:C16_WA + NT * E]
        w_carB = c16_sb[:, C16_WB:C16_WB + NT * E]
        exp8 = c16_sb[:, C16_EXP8:C16_EXP8 + E]
        iotac = cf_sb[:, CF_IOTAC:CF_IOTAC + NT]
        iotaw = cf_sb[:, CF_IOTAW:CF_IOTAW + RT * 8]
        wr32 = cf_sb[:, CF_WR:CF_WR + DC * E]
        iotaE = cf_sb[:, CF_IOTAE:CF_IOTAE + TPB * E]

        with (
            tc.tile_pool(name="persist", bufs=1) as pp,
            tc.tile_pool(name="dramb", bufs=1, space="DRAM") as dram,
            tc.tile_pool(name="w1pool", bufs=4) as wp,
        ):
            m_all = pp.tile([P, NT, E], f32, tag="mall")
            m16 = pp.tile([P, NT, E], f16, tag="m16")
            xT_all = pp.tile([P, DC, CAP], f16, tag="xTall")
            hT_all = pp.tile([P, JG, FCAP], f16, tag="hTall")
            o16 = pp.tile([P, KC, FCAP], f16, tag="o16")
            s2ro = pp.tile([P, RT, REC], f32, tag="s2ro")
            w2_all = pp.tile([P, JG, D_OUT], f16, tag="w2all")
            sc_big = pp.tile([P, NT, 2], f32, tag="scbig")
            tok_s8 = pp.tile([1, 1], f16, tag="toks8")   # weight-chain token
            tok_xg = pp.tile([1, 1], f16, tag="tokxg")   # weight-chain token
            w1_tiles = []
            bounce_in = dram.tile([P, E], f16, tag="bin")
            bounce_out = dram.tile([E, P, E], f16, tag="bout")

            # scatter records: col0 = token id (known now), col1 = score
            nc.vector.tensor_copy(out=sc_big[:, :, 0], in_=iotac)

            # ---------------- router: own 512-token slice, exact fp32 --------
            # logits[tok, e] with x chunks as the (free) PE weights and the
            # tiny Wr as the moving operand. c-outer with 4 concurrent PSUM
            # accumulators so the PE trails the 4-chunk x DMA stream.
            with (
                tc.tile_pool(name="rwork", bufs=1) as rp,
                tc.tile_pool(name="tpsum", bufs=1, space="PSUM") as tps,
            ):
                xtt = rp.tile([P, DC, BT], f32, tag="xtt")
                for cc in range(4):
                    nc.sync.dma_start(
                        xtt[:, 2 * cc:2 * cc + 2, :], x32s[:, 2 * cc:2 * cc + 2, :]
                    )
                # remaining consts ride the sync queue behind the x slice
                nc.sync.dma_start(c16_sb[:], c16)
                nc.sync.dma_start(crp_sb[:], crp)
                nc.sync.dma_start(selg_sb[:], io["selg"])

                lg_ps = []
                for i in range(TPB):
                    lg_ps_i = tps.tile([P, 8], f32, tag=f"lgps{i}", name=f"lgps{i}")
                    lg_ps.append(lg_ps_i)
                for c in range(DC):
                    for i in range(TPB):
                        nc.tensor.matmul(
                            lg_ps[i][:],
                            lhsT=xtt[:, c, i * P:(i + 1) * P],
                            rhs=wr32[:].rearrange("p (c e) -> p c e", c=DC)[:, c, :],
                            start=(c == 0), stop=(c == DC - 1),
                        )
                lg_sl = rp.tile([P, TPB, E], f32, tag="lgsl")
                for i in range(TPB):
                    nc.vector.tensor_copy(out=lg_sl[:, i, :], in_=lg_ps[i][:])

                pk = rp.tile([P, 2 * TPB], f16, tag="pk")
                mx_sl = rp.tile([P, TPB], f32, tag="mxsl")
                nc.vector.tensor_reduce(
                    out=mx_sl[:], in_=lg_sl[:], axis=Ax.X, op=Alu.max
                )
                # renc = E - argmax (first-max tiebreak) via one priority max:
                # max_e(lg*2^20 + (E-e)) - max_e(lg)*2^20, exact in fp32.
                # Runs on the Pool engine so the score path (DVE) overlaps.
                sc1 = rp.tile([P, TPB, E], f32, tag="sc1")
                nc.gpsimd.tensor_scalar(
                    out=sc1[:], in0=lg_sl[:], scalar1=PSCALE, scalar2=None,
                    op0=Alu.mult,
                )
                sc2 = rp.tile([P, TPB, E], f32, tag="sc2")
                nc.gpsimd.tensor_tensor(
                    out=sc2[:].rearrange("p t e -> p (t e)"),
                    in0=sc1[:].rearrange("p t e -> p (t e)"),
                    in1=iotaE, op=Alu.add,
                )
                vmax = rp.tile([P, TPB], f32, tag="vmax")
                nc.vector.tensor_reduce(
                    out=vmax[:], in_=sc2[:], axis=Ax.X, op=Alu.max
                )
                mxs = rp.tile([P, TPB], f32, tag="mxs")
                nc.gpsimd.tensor_scalar(
                    out=mxs[:], in0=mx_sl[:], scalar1=PSCALE, scalar2=None,
                    op0=Alu.mult,
                )
                nc.gpsimd.tensor_tensor(
                    out=pk[:, 0:TPB], in0=vmax[:], in1=mxs[:], op=Alu.subtract,
                )
                # score = exp(mx) / sum_e exp(lg)
                el = rp.tile([P, TPB, E], f32, tag="el")
                nc.scalar.activation(el[:], lg_sl[:], Act.Exp)
                ssum = rp.tile([P, TPB], f32, tag="ssum")
                nc.vector.tensor_reduce(
                    out=ssum[:], in_=el[:], axis=Ax.X, op=Alu.add
                )
                emx = rp.tile([P, TPB], f32, tag="emx")
                nc.scalar.activation(emx[:], mx_sl[:], Act.Exp)
                rsum = rp.tile([P, TPB], f32, tag="rsum")
                nc.vector.reciprocal(rsum[:], ssum[:])
                nc.vector.tensor_tensor(
                    out=pk[:, TPB:2 * TPB], in0=emx[:], in1=rsum[:],
                    op=Alu.mult,
                )
                # ship pk [P, 8] as-is; the AllGather stacks cores on axis 0
                nc.sync.dma_start(bounce_in[:], pk[:])
                # weight-stream chain token: W1 block 0 may start now.
                # Gate on the score half (written last) so w1b0 cannot beat
                # the bounce write to the DMA mutex.
                nc.scalar.activation(
                    tok_s8[:], pk[0:1, TPB:TPB + 1], Act.Copy
                )
                # first three W1 blocks: emitted here (early Act-queue slots)
                # so the paced chain fills the dead DMA window under the
                # collective; blocks 3+ ride the chain during L1
                for jb in range(3):
                    w1b = wp.tile([P, DC, JB], f16, tag="w1b", name=f"w1b{jb}")
                    nc.scalar.activation(
                        w1b[0:1, 0:1, 0:1],
                        tok_s8[:] if jb == 0 else w1_tiles[-1][0:1, 0:1, 0:1],
                        Act.Copy,
                    )
                    nc.sync.dma_start(w1b[:], w1t[:, jb])
                    w1_tiles.append(w1b)

                # broadcast routing decisions: [8, 128] -> [64, 128]
                nc.gpsimd.collective_compute(
                    "AllGather",
                    Alu.bypass,
                    replica_groups=[list(range(E))],
                    ins=[bounce_in.opt()],
                    outs=[bounce_out.opt()],
                )

                # reload [s, p, j] -> SBUF [p, s, j] (small strided DMA):
                # ld_v[p, s, j] holds renc (j<4, t-tile 4s+j) and score
                # (j>=4, t-tile 4s+j-4)
                ld_t = rp.tile([P, E, E], f16, tag="ldt")
                nc.sync.dma_start(
                    ld_t[:], bounce_out[:].rearrange("s p j -> p s j")
                )
                ld_v = ld_t[:]
                renc_v = ld_v[:, :, 0:TPB]      # [P, s, j] == [P, t=4s+j]
                nc.vector.tensor_copy(
                    out=sc_big[:].rearrange("p t c -> p t c")[:, :, 1].rearrange(
                        "p (s j) -> p s j", s=E),
                    in_=ld_v[:, :, TPB:2 * TPB],
                )
                # sort masks for all 32 t-tiles: f16 (PE operand) on DVE,
                # f32 (DVE operand) on the Pool engine, in parallel
                for e in range(E):
                    nc.vector.tensor_scalar(
                        out=m16[:, :, e].rearrange("p (s j) -> p s j", s=E),
                        in0=renc_v,
                        scalar1=float(E - e), scalar2=None,
                        op0=Alu.is_equal,
                    )
                for e in range(E):
                    nc.gpsimd.tensor_scalar(
                        out=m_all[:, :, e].rearrange("p (s j) -> p s j", s=E),
                        in0=renc_v,
                        scalar1=float(E - e), scalar2=None,
                        op0=Alu.is_equal,
                    )

            # ---------------- stable counting sort ----------------
            # dest[p,t] = sum_e m_e * (within-tile prefix over p + carry)
            # carry[(te)] = sum_{t'e'} cs[(t'e')] * ([e'<e] + [e'==e][t'<t])
            # computed as two PE matmuls on the partition-major colsums.
            m16f = m16[:].rearrange("p t e -> p (t e)")
            with (
                tc.tile_pool(name="swork", bufs=1) as sw,
            ):
                csA_bc = sw.tile([P, P], f16, tag="csAbc")
                csB_bc = sw.tile([P, P], f16, tag="csBbc")
                posw = sw.tile([P, RT * 8], f32, tag="posw")
                pos16w = sw.tile([P, RT * 8], i16, tag="pos16w")
                with tc.tile_pool(name="spsA", bufs=1, space="PSUM") as spsA:
                    csA_ps = spsA.tile([P, 1], f32, tag="csA")
                    nc.tensor.matmul(
                        csA_ps[:], lhsT=m16f[:, 0:P], rhs=ones1_16,
                        start=True, stop=True,
                    )
                    csB_ps = spsA.tile([P, 1], f32, tag="csB")
                    nc.tensor.matmul(
                        csB_ps[:], lhsT=m16f[:, P:2 * P], rhs=ones1_16,
                        start=True, stop=True,
                    )
                    nc.vector.tensor_scalar(
                        out=csA_bc[:], in0=ones128, scalar1=csA_ps[:, 0:1],
                        scalar2=None, op0=Alu.mult,
                    )
                    nc.vector.tensor_scalar(
                        out=csB_bc[:], in0=ones128, scalar1=csB_ps[:, 0:1],
                        scalar2=None, op0=Alu.mult,
                    )
                    # own expert's base offset, broadcast down partitions
                    own_ps = spsA.tile([P, 1], f32, tag="own")
                    nc.tensor.matmul(
                        own_ps[:], lhsT=csA_bc[:], rhs=crp_sb[:],
                        start=True, stop=False,
                    )
                    nc.tensor.matmul(
                        own_ps[:], lhsT=csB_bc[:], rhs=crp_sb[:],
                        start=False, stop=True,
                    )
                    # own sorted positions, wrapped layout (pre-computed so
                    # the sgo gather can fire the moment the scatter lands)
                    nc.vector.tensor_scalar(
                        out=posw[:], in0=iotaw,
                        scalar1=own_ps[:, 0:1], scalar2=float(N_TOKENS - 1),
                        op0=Alu.add, op1=Alu.min,
                    )
                    nc.vector.tensor_copy(out=pos16w[:], in_=posw[:])

                dest = sw.tile([P, NT], f32, tag="dest")
                with tc.tile_pool(name="spsB", bufs=1, space="PSUM") as spsB:
                    prefix_ps = spsB.tile([P, NT, E], f32, tag="prefix")
                    nc.tensor.matmul(
                        prefix_ps[:].rearrange("p t e -> p (t e)"),
                        lhsT=tri16, rhs=m16f, start=True, stop=True,
                    )
                    carb_ps = spsB.tile([P, NT * E], f32, tag="carb")
                    nc.tensor.matmul(
                        carb_ps[:], lhsT=csA_bc[:], rhs=w_carA,
                        start=True, stop=False,
                    )
                    nc.tensor.matmul(
                        carb_ps[:], lhsT=csB_bc[:], rhs=w_carB,
                        start=False, stop=True,
                    )
                    # (hardware allows only one PSUM input per vector op)
                    carb_sb = sw.tile([P, NT, E], f32, tag="carbsb")
                    nc.vector.tensor_copy(
                        out=carb_sb[:].rearrange("p t e -> p (t e)"),
                        in_=carb_ps[:],
                    )
                    s1 = sw.tile([P, NT, E], f32, tag="s1")
                    nc.vector.tensor_tensor(
                        out=s1[:], in0=prefix_ps[:], in1=carb_sb[:], op=Alu.add
                    )
                    s2 = sw.tile([P, NT, E], f32, tag="s2")
                    nc.vector.tensor_tensor(
                        out=s2[:], in0=s1[:], in1=m_all[:], op=Alu.mult
                    )
                    nc.vector.tensor_reduce(
                        out=dest[:], in_=s2[:], axis=Ax.X, op=Alu.add
                    )

                # wrap + replicate scatter indices on the PE: slot i = t*128+p
                # lives at [i%16 (+16c), t*8 + p//16]; the selection matmul
                # moves dest[16g + q%16, t] to partition q, column group g,
                # replicated for all 8 Q7 cores at once.
                dest16w = sw.tile([P, NT, 8], i16, tag="dest16w")
                with tc.tile_pool(name="wps", bufs=1, space="PSUM") as wps:
                    wp_ps = wps.tile([P, 8, NT], f32, tag="wpps")
                    for g in range(8):
                        nc.tensor.matmul(
                            wp_ps[:, g, :], lhsT=selg_sb[:, g, :], rhs=dest[:],
                            start=True, stop=True,
                        )
                    nc.vector.tensor_copy(
                        out=dest16w[:], in_=wp_ps[:].rearrange("p g t -> p t g")
                    )
                nc.gpsimd.dma_scatter_add(
                    sidx[:, 0:2], sc_big[:],
                    dest16w[:].rearrange("p t g -> p (t g)"),
                    N_TOKENS, N_TOKENS, 2, elem_step=REC,
                )

                # expert counts for the host combine (off the critical path)
                with tc.tile_pool(name="cntp", bufs=1, space="PSUM") as cntp:
                    cnt_ps = cntp.tile([P, E], f32, tag="cntps")
                    nc.tensor.matmul(
                        cnt_ps[:], lhsT=csA_bc[:], rhs=exp8,
                        start=True, stop=False,
                    )
                    nc.tensor.matmul(
                        cnt_ps[:], lhsT=csB_bc[:], rhs=exp8,
                        start=False, stop=True,
                    )
                    cnt_row = sw.tile([1, E], f32, tag="cnt")
                    nc.vector.tensor_copy(out=cnt_row[:], in_=cnt_ps[0:1, :])
                nc.scalar.dma_start(cnts, cnt_row[:])

                # ---------------- gather own rows + scaled transpose ---------
                sgo = sw.tile([P, RT, REC], f32, tag="sgo")
                nc.gpsimd.dma_gather(
                    sgo[:], sidx, pos16w[:], CAP, CAP, REC,
                )
                ids16w = sw.tile([P, RT, 8], i16, tag="ids16w")
                with tc.tile_pool(name="iwps", bufs=1, space="PSUM") as iwps:
                    iw_ps = iwps.tile([P, 8, RT], f32, tag="iwps")
                    for g in range(8):
                        nc.tensor.matmul(
                            iw_ps[:, g, :], lhsT=selg_sb[:, g, :],
                            rhs=sgo[:, :, 0], start=True, stop=True,
                        )
                    nc.vector.tensor_copy(
                        out=ids16w[:], in_=iw_ps[:].rearrange("p g r -> p r g")
                    )
                # PE p-state warmup: keep the tensor engine busy through the
                # xg-gather window so the transpose + L1 streams start at max
                # clock instead of re-ramping from cold (results unused)
                with tc.tile_pool(name="wmps", bufs=1, space="PSUM") as wmps:
                    scr = wmps.tile([P, P], f32, tag="warm")
                    for _ in range(48):
                        nc.tensor.matmul(
                            scr[:], lhsT=ident16, rhs=ident16,
                            start=True, stop=True,
                        )
                idsw_flat = ids16w[:].rearrange("p r g -> p (r g)")
                # scale lookup first (small; unblocks the diag tiles early):
                # sorted_scores[token_id]
                nc.gpsimd.dma_gather(
                    s2ro[:], sidx, idsw_flat, CAP, CAP, REC,
                )
                xg = sw.tile([P, RT, D_IN], f16, tag="xg")
                nc.gpsimd.dma_gather(
                    xg[:, 0:3, :], x16, idsw_flat[:, 0:24], 384, 384, D_IN,
                )
                nc.gpsimd.dma_gather(
                    xg[:, 3:RT, :], x16, idsw_flat[:, 24:40], 256, 256, D_IN,
                )
                # weight-stream chain token: resume the stream past xg
                nc.scalar.activation(tok_xg[:], xg[0:1, 0:1, 0:1], Act.Copy)
                nc.scalar.dma_start(ids5, sgo[:, :, 0])
                # transpose gathered rows, folding the positional score scale
                # in by multiplying with diag(score) on the PE
                # (exact because b1 = b2 = 0 and scores > 0).
                # All 5 diag tiles are built up front so the transpose matmul
                # stream never stalls on a diag between row tiles.
                diag_all = sw.tile([P, RT, P], f16, tag="diagall")
                for rt in range(RT):
                    nc.vector.tensor_scalar(
                        out=diag_all[:, rt, :], in0=ident16,
                        scalar1=s2ro[:, rt, 1:2], scalar2=None, op0=Alu.mult,
                    )
                with tc.tile_pool(name="tpx", bufs=3, space="PSUM") as tpx:
                    for rt in range(RT):
                        diag = diag_all[:, rt, :]
                        tp = tpx.tile([P, DC, P], f32, tag="tp")
                        for c in range(DC):
                            nc.tensor.matmul(
                                tp[:, c, :],
                                lhsT=xg[:, rt, c * P:(c + 1) * P],
                                rhs=diag,
                                start=True, stop=True,
                            )
                        # one batched drain per row tile, split DVE/Act
                        nc.vector.tensor_copy(
                            out=xT_all[:, 0:DC // 2, rt * P:(rt + 1) * P],
                            in_=tp[:, 0:DC // 2, :],
                        )
                        nc.scalar.activation(
                            xT_all[:, DC // 2:DC, rt * P:(rt + 1) * P],
                            tp[:, DC // 2:DC, :], Act.Copy,
                        )
                    # bridge the sem gap between the transposes and L1's
                    # first matmul so the PE stays at max clock
                    warm_tp = tpx.tile([P, DC, P], f32, tag="tp")
                    for _ in range(24):
                        nc.tensor.matmul(
                            warm_tp[:, 0, :], lhsT=ident16, rhs=ident16,
                            start=True, stop=True,
                        )

                # W2: four 2MB DMAs on the Pool queue, chained via DVE corner
                # copies (each chunk gated on the previous) so W1 blocks can
                # interleave at the DMA mutex between chunks. Chunk 0 is
                # gated on the xg token so nothing hoists ahead of the spine.
                w2v = w2t[:].rearrange("p a b k -> p (a b) k")
                for q in range(4):
                    nc.vector.tensor_copy(
                        out=w2_all[0:1, 8 * q:8 * q + 1, 0:1],
                        in_=tok_xg[:] if q == 0
                        else w2_all[0:1, 8 * (q - 1):8 * (q - 1) + 1, 0:1],
                    )
                    nc.gpsimd.dma_start(
                        w2_all[:, 8 * q:8 * (q + 1), :],
                        w2v[:, 8 * q:8 * (q + 1), :],
                    )

            # ---------------- FFN layer 1 (paced W1 stream) ------------------
            # c-outer so both token chunks reuse the stationary W1 tile
            with (
                tc.tile_pool(name="l1ps", bufs=3, space="PSUM") as l1ps,
                tc.tile_pool(name="l1tail", bufs=3, space="PSUM") as l1tail,
            ):
                for jb in range(NJB):
                    if jb < 3:
                        # prefetched before the collective (paced chain head)
                        w1b = w1_tiles[jb]
                    else:
                        w1b = wp.tile([P, DC, JB], f16, tag="w1b",
                                      name=f"w1b{jb}")
                        # paced chain: block jb's DMA is gated on block jb-1
                        # (and the xg-pause token) via corner copies so at
                        # most one weight chunk contends with the spine DMAs
                        nc.scalar.activation(
                            w1b[0:1, 0:1, 0:1],
                            w1_tiles[-1][0:1, 0:1, 0:1], Act.Copy,
                        )
                        if jb == 3:
                            nc.scalar.activation(
                                w1b[0:1, 0:2, 0:1].rearrange(
                                    "p c j -> p (c j)")[:, 1:2],
                                tok_xg[:], Act.Copy,
                            )
                        nc.sync.dma_start(w1b[:], w1t[:, jb])
                        w1_tiles.append(w1b)
                    for jc in range(JCB):
                        jg = jb * JCB + jc
                        ps_a = l1ps.tile([P, 512], f32, tag="l1pa")
                        ps_b = l1tail.tile([P, FCAP - 512], f32, tag="l1pb")
                        # chunk A first: it only needs row tiles 0-3, so the
                        # first matmuls can fire before rt4's transpose lands
                        # (Ldweights is pipelined-free, so re-loading the
                        # stationary for chunk B costs nothing)
                        for c in range(DC):
                            nc.tensor.matmul(
                                ps_a[:], lhsT=w1b[:, c, jc * P:(jc + 1) * P],
                                rhs=xT_all[:, c, 0:512],
                                start=(c == 0), stop=(c == DC - 1),
                            )
                        for c in range(DC):
                            nc.tensor.matmul(
                                ps_b[:], lhsT=w1b[:, c, jc * P:(jc + 1) * P],
                                rhs=xT_all[:, c, 512:FCAP],
                                start=(c == 0), stop=(c == DC - 1),
                            )
                        nc.scalar.activation(
                            hT_all[:, jg, 0:512], ps_a[:], Act.Relu
                        )
                        nc.scalar.activation(
                            hT_all[:, jg, 512:FCAP], ps_b[:], Act.Relu
                        )
            # ---------------- FFN layer 2 (full PSUM accumulation) -----------
            # g-outer with both chunks inner: one Ldweights per (g, kc)
            with (
                tc.tile_pool(name="l2ps", bufs=2, space="PSUM") as l2ps,
                tc.tile_pool(name="l2tail", bufs=2, space="PSUM") as l2tail,
            ):
                for kc in range(KC):
                    ps_a = l2ps.tile([P, 512], f32, tag="l2pa")
                    ps_b = l2tail.tile([P, FCAP - 512], f32, tag="l2pb")
                    for g in range(JG):
                        lhsT = w2_all[:, g, kc * P:(kc + 1) * P]
                        nc.tensor.matmul(
                            ps_a[:], lhsT=lhsT, rhs=hT_all[:, g, 0:512],
                            start=(g == 0), stop=(g == JG - 1),
                        )
                        nc.tensor.matmul(
                            ps_b[:], lhsT=lhsT, rhs=hT_all[:, g, 512:FCAP],
                            start=(g == 0), stop=(g == JG - 1),
                        )
                    if kc < KC - 1:
                        nc.vector.tensor_copy(
                            out=o16[:, kc, 0:512], in_=ps_a[:]
                        )
                        nc.scalar.activation(
                            o16[:, kc, 512:FCAP], ps_b[:], Act.Copy
                        )
                        nc.sync.dma_start(outT16[:, kc, :], o16[:, kc, :])
                    else:
                        # last tile: drain across three engines and ship each
                        # piece as soon as its copy lands to shorten the tail
                        nc.vector.tensor_copy(
                            out=o16[:, kc, 0:256], in_=ps_a[:, 0:256]
                        )
                        nc.scalar.activation(
                            o16[:, kc, 256:512], ps_a[:, 256:512], Act.Copy
                        )
                        nc.scalar.activation(
                            o16[:, kc, 512:FCAP], ps_b[:], Act.Copy
                        )
                        nc.sync.dma_start(
                            outT16[:, kc, 0:256], o16[:, kc, 0:256]
                        )
                        nc.sync.dma_start(
                            outT16[:, kc, 256:512], o16[:, kc, 256:512]
                        )
                        nc.sync.dma_start(
                            outT16[:, kc, 512:FCAP], o16[:, kc, 512:FCAP]
                        )


_NC_CACHE = None


def _get_nc():
    global _NC_CACHE
    if _NC_CACHE is None:
        _NC_CACHE = build_nc()
    return _NC_CACHE


def _make_in_maps(x, Wr, br, W1, b1, W2, b2):
    x = np.asarray(x, np.float32)
    Wr = np.asarray(Wr, np.float32)
    br = np.asarray(br, np.float32)
    W1 = np.asarray(W1, np.float32)
    W2 = np.asarray(W2, np.float32)
    b1 = np.asarray(b1, np.float32)
    b2 = np.asarray(b2, np.float32)
    # the kernel folds the positional score scale onto x and drops the FFN
    # bias adds, which is exact only for zero biases (the spec generates
    # zeros)
    assert not np.any(b1) and not np.any(b2), "nonzero FFN biases unsupported"
    assert not np.any(br), "nonzero router bias unsupported"

    x16 = x.astype(np.float16)
    # x32s[e][p, c, n] = x[e*512 + n, c*128 + p]  (fp32: exact router)
    x_resh = x.reshape(E, BT, DC, P)

    p = np.arange(P)
    c16 = np.zeros((P, C16_W), np.float16)
    c16[:, C16_IDENT:C16_IDENT + P] = np.eye(P, dtype=np.float16)
    c16[:, C16_TRI:C16_TRI + P] = (p[:, None] < p[None, :]).astype(np.float16)
    c16[:, C16_ONES] = 1.0
    # AllGather row selectors: row 8s+j is renc t-tile 4s+j, row 8s+4+j is
    # score t-tile 4s+j
    for s in range(E):
        for j in range(TPB):
            c16[8 * s + j, C16_SELR + 4 * s + j] = 1.0
            c16[8 * s + 4 + j, C16_SELS + 4 * s + j] = 1.0
    c16[:, C16_O128:C16_O128 + P] = 1.0
    # carry matrices: row k=(t'*8+e'), col (t*8+e):
    #   W[(t'e'),(te)] = [e'<e] + [e'==e][t'<t]
    te = np.arange(NT * E)
    t_of, e_of = te // E, te % E
    for half, base in ((0, C16_WA), (1, C16_WB)):
        k = np.arange(P)
        tk, ek = k // E + 16 * half, k % E
        mat = (ek[:, None] < e_of[None, :]).astype(np.float16)
        mat += (ek[:, None] == e_of[None, :]) & (tk[:, None] < t_of[None, :])
        c16[:, base:base + NT * E] = mat.astype(np.float16)
    c16[:, C16_EXP8:C16_EXP8 + E] = (
        (np.arange(P) % E)[:, None] == np.arange(E)[None, :]
    ).astype(np.float16)

    cf32 = np.zeros((P, CF_W), np.float32)
    cf32[:, CF_IOTAC:CF_IOTAC + NT] = (
        np.arange(NT)[None, :] * P + p[:, None]
    ).astype(np.float32)
    cf32[:, CF_IOTAW:CF_IOTAW + RT * 8] = (
        np.arange(RT * 8)[None, :] * 16 + (p % 16)[:, None]
    ).astype(np.float32)
    cf32[:, CF_WR:CF_WR + DC * E] = (
        Wr.reshape(DC, P, E).transpose(1, 0, 2).reshape(P, DC * E)
    )
    cf32[:, CF_IOTAE:CF_IOTAE + TPB * E] = (
        float(E) - (np.arange(TPB * E) % E)[None, :]
    ).astype(np.float32)

    sidx0 = np.zeros((N_TOKENS, REC), np.float32)
    selg = np.zeros((P, 8, P), np.float32)
    g_idx = np.arange(8)
    for pp in range(P):
        selg[g_idx * 16 + (pp % 16), g_idx, pp] = 1.0

    shared = dict(
        x16=np.ascontiguousarray(x16), c16=c16, cf32=cf32,
        sidx=sidx0, selg=selg,
    )

    in_maps = []
    for e in range(E):
        m = dict(shared)
        m["x32s"] = np.ascontiguousarray(x_resh[e].transpose(2, 1, 0))
        w1e = W1[e].astype(np.float16)
        # w1t[p, jb, c, j] = W1[c*128 + p, jb*512 + j]
        m["w1t"] = np.ascontiguousarray(
            w1e.reshape(DC, P, NJB, JB).transpose(1, 2, 0, 3)
        )
        w2e = W2[e].astype(np.float16)
        # w2t[p, wb, g, k] = W2[(wb*4 + g)*128 + p, k]
        m["w2t"] = np.ascontiguousarray(
            w2e.reshape(NWB, JG // NWB, P, D_OUT).transpose(2, 0, 1, 3)
        )
        # crp[k, 0] = [k%8 < e]  (same pattern both halves of (t'e') rows)
        m["crp"] = ((np.arange(P) % E) < e).astype(np.float16)[:, None]
        in_maps.append(m)
    return in_maps


def _combine(results):
    out = np.zeros((N_TOKENS, D_OUT), np.float32)
    cnts = results[0]["cnts"][0]
    total = 0
    for e in range(E):
        n = int(round(float(cnts[e])))
        assert 0 <= n <= FCAP, f"expert {e} count {n} exceeds capacity {FCAP}"
        idx = results[e]["ids5"].T.reshape(CAP)[:n].astype(np.int64)
        arr = results[e]["outT16"].reshape(P, KC, FCAP)
        rows = np.transpose(arr, (2, 1, 0)).reshape(FCAP, KC * P).astype(np.float32)
        out[idx] = rows[:n]
        total += n
    assert total == N_TOKENS, f"token counts sum to {total}, expected {N_TOKENS}"
    return out


def kernel(**inputs) -> np.ndarray:
    nc = _get_nc()
    in_maps = _make_in_maps(**inputs)
    res = run_bass_kernel_spmd(nc, in_maps, core_ids=list(range(E)))
    return _combine(res.results)


def kernel_traced(**inputs):
    """Like kernel() but with NTFF profiling; returns (out, BassKernelResults)."""
    nc = _get_nc()
    in_maps = _make_in_maps(**inputs)
    res = run_bass_kernel_spmd(
        nc, in_maps, core_ids=list(range(E)), trace=True
    )
    return _combine(res.results), res
